# revision 49
# baseline (speedup 1.0000x reference)
"""Trainium2 Bass kernel for nn_Attention_51548197487430.

Multi-head attention (B=2, S=2048, D=1024, H=16, HD=64), fp32 reference,
sharded 2 heads per core across 8 NeuronCores (head/tensor parallel per
the sharding hint: w_qkv output dim and w_out input dim split per-head;
the all-reduce after the output projection is realized as the host-side
unshard step, which sums the 8 partial outputs).

All matmuls bf16/fp16 (fp8 paths tested and rejected: each one alone
exceeds the 2e-2 error budget on these inputs).  The schedule is built
to keep the PE saturated -- its ~171us of matmul work is the roofline:

  - warmup matmuls ramp the HAM clock gate while the first DMA lands;
    the first x chunk is d-sliced so the QKV projection starts ~1us in.
  - all 8 x^T chunk DMAs are issued upfront (64KB/partition of SBUF).
  - attention k-loop emits scores(k+1) BEFORE attnV(k), so the PE queue
    always has independent work ahead of the exp dependency and
    LDWEIGHTS prefetch is never blocked by a semaphore wait at the
    queue head.
  - dedicated PSUM pools: scores 2x[128,1024] (4 banks), attnV
    accumulators psA/psB (2 banks), utility rotation (2 banks).
  - softmax normalize entirely on DVE via partition-base-shifted
    reciprocals (no PE broadcast matmuls).
  - out-projection in fp16, pipelined into the next iteration's k-loop
    as PE filler; PSUM->SBUF drains on DVE; ACT does exp only.
"""

import numpy as np

B, S, D, H, HD = 2, 2048, 1024, 16, 64
N_CORES = 8
SCALE = HD ** (-0.5)
BS = B * S               # 4096
SC = 512                 # qkv-phase s-chunk (8 chunks)
QC = 512                 # attention q-chunk
NKC = S // 128           # 16 k-chunks per batch
DC = D // 128            # 8 contraction chunks
WARMUP = 10              # HAM-ramp matmuls at t=0, spanning the DMA wait

_cache = {}


def _build():
    import concourse.bass as bass
    import concourse.mybir as mybir
    import concourse.tile as tile
    from concourse import bacc

    F32 = mybir.dt.float32
    F32R = mybir.dt.float32r
    BF16 = mybir.dt.bfloat16
    F16 = mybir.dt.float16
    AF = mybir.ActivationFunctionType

    nc = bacc.Bacc("TRN2", target_bir_lowering=False, debug=False,
                   num_devices=N_CORES)
    # host pre-arranges x^T and w_qkv^T into per-partition-contiguous tile
    # layouts so every DMA is 128 large contiguous descriptors
    xT_d = nc.dram_tensor("xT", (128, BS // SC, DC, SC), BF16,
                          kind="ExternalInput").ap()
    wqkvT_d = nc.dram_tensor("wqkvT", (128, DC, 384), BF16,
                             kind="ExternalInput").ap()
    woutT_d = nc.dram_tensor("woutT", (128, D), F16, kind="ExternalInput").ap()
    # fp16 partials: |values| << 1, so fp16's 10-bit mantissa beats bf16
    # and halves the output DMA; host upcasts and sums in fp32.
    out_d = nc.dram_tensor("out", (BS, D), F16, kind="ExternalOutput").ap()

    with tile.TileContext(nc) as tc:
        with tc.tile_pool(name="persist", bufs=1) as persist, \
             tc.tile_pool(name="xin", bufs=8) as xin, \
             tc.tile_pool(name="epool", bufs=10) as epool, \
             tc.tile_pool(name="work", bufs=4) as work, \
             tc.tile_pool(name="osbp", bufs=5) as osbp, \
             tc.tile_pool(name="ps_sc", bufs=2, space="PSUM") as ps_sc, \
             tc.tile_pool(name="ps_acc", bufs=1, space="PSUM") as ps_acc, \
             tc.tile_pool(name="ps_util", bufs=2, space="PSUM") as ps_util:

            # ---- persistent tiles ----
            warm = persist.tile([128, SC], BF16, tag="warm")
            nc.gpsimd.memset(warm[:], 0.0)

            wqkvT = persist.tile([128, DC, 384], BF16, tag="wqkvT")
            nc.sync.dma_start(wqkvT[:], wqkvT_d)
            woutT = persist.tile([128, D], F16, tag="woutT")
            nc.sync.dma_start(woutT[:], woutT_d)

            identb = persist.tile([128, 128], BF16, tag="identb")
            from concourse.masks import make_identity
            make_identity(nc, identb[:])

            QT = persist.tile([128, BS], BF16, tag="QT")
            KT = persist.tile([128, BS], BF16, tag="KT")
            VT = persist.tile([128, BS], BF16, tag="VT")
            # V_aug[b][h]: (128, NKC, 128); h0 = [V | ones], h1 = [ones | V]
            # the ones-columns compute the softmax denominator inside the
            # same attn@V matmul (PE cost ~ N only, so the rows are free).
            vaug = [[persist.tile([128, NKC, 128], BF16, tag=f"vaug{b}{h}",
                                  name=f"vaug{b}{h}")
                     for h in range(2)] for b in range(B)]
            const_f32 = persist.tile([128, NKC * 64], F32, tag="const_f32")
            nc.gpsimd.memset(const_f32[:], 1.0)
            ones_3d = const_f32[:].rearrange("p (a b) -> p a b", b=64)
            for b in range(B):
                nc.vector.tensor_copy(vaug[b][0][:, :, 64:128], ones_3d)
                nc.vector.tensor_copy(vaug[b][1][:, :, 0:64], ones_3d)
            # inv2: anti-block-diagonal 1/64 weights; one matmul pair
            # accumulates both heads' denominator broadcasts into ONE psum
            # tile (h0's denom -> partitions 0-63, h1's -> 64-127).
            inv2 = persist.tile([128, 128], F32R, tag="inv2")
            inv2_f32 = persist.tile([128, 128], F32, tag="inv2_f32")
            nc.gpsimd.memset(inv2_f32[:], 0.0)
            nc.gpsimd.memset(inv2_f32[64:128, 0:64], 1.0 / 64.0)
            nc.gpsimd.memset(inv2_f32[0:64, 64:128], 1.0 / 64.0)
            nc.vector.tensor_copy(inv2[:], inv2_f32[:])

            # ---- all xT chunk DMAs upfront; chunk 0 d-sliced for fast start
            # every chunk d-sliced: 64 independent DMAs spread round-robin
            # across the 16 queues so each lands in a few us
            xts = []
            for s in range(BS // SC):
                xt = xin.tile([128, DC, SC], BF16, tag="xt", name=f"xt{s}")
                for d in range(DC):
                    nc.sync.dma_start(xt[:, d, :], xT_d[:, s, d, :])
                xts.append(xt)

            # ---- warmup: ramp the PE clock while the first DMAs land ----
            for _ in range(WARMUP):
                pw = ps_util.tile([128, SC], F32, tag="pu", name="warm_ps")
                nc.tensor.matmul(pw[:], lhsT=warm[:, 0:128], rhs=warm[:],
                                 start=True, stop=True)

            def emit_qkv_part(s, e):
                """One e-chunk (Q, K or V) of the projection for s-chunk s."""
                dst = (QT, KT, VT)[e]
                ps = ps_util.tile([128, SC], F32, tag="pu", name="qkv_ps")
                for d in range(DC):
                    nc.tensor.matmul(
                        ps[:], lhsT=wqkvT[:, d, 128 * e:128 * (e + 1)],
                        rhs=xts[s][:, d, :], start=(d == 0), stop=(d == DC - 1))
                nc.vector.tensor_copy(dst[:, s * SC:(s + 1) * SC], ps[:])

            def emit_vtrans(s):
                """Transpose s-chunk s of V^T into the vaug tiles."""
                b = s // 4
                k0 = (s % 4) * 4       # first k-chunk (within batch)
                pt = ps_util.tile([128, 4, 128], BF16, tag="pu", name="vt_ps")
                for i in range(4):
                    j = s * 4 + i      # global 128-col index
                    nc.tensor.transpose(pt[:, i, :],
                                        VT[:, j * 128:(j + 1) * 128], identb[:])
                nc.vector.tensor_copy(vaug[b][0][:, k0:k0 + 4, 0:64],
                                      pt[:, 0:4, 0:64])
                nc.vector.tensor_copy(vaug[b][1][:, k0:k0 + 4, 64:128],
                                      pt[:, 0:4, 64:128])

            def emit_finish_stage(st, stage):
                """Software-pipelined tail of the previous attention
                iteration, interleaved into the current k-loop."""
                if st is None:
                    return
                if stage == 0:
                    # odA = [o_h0 | denomA], odB = [denomB | o_h1]
                    st["odA"] = work.tile([128, QC], F32R, tag="odA", name="odA")
                    st["odB"] = work.tile([128, QC], F32R, tag="odB", name="odB")
                    nc.vector.tensor_copy(st["odA"][:], st["psA"][:])
                    nc.vector.tensor_copy(st["odB"][:], st["psB"][:])
                elif stage == 1:
                    # both heads' denominators broadcast into ONE psum tile
                    # via the anti-block-diagonal inv2, then one fast
                    # reciprocal (base-0 full tile: custom-DVE ops NaN on
                    # partition-base-shifted APs)
                    st["pbc"] = ps_util.tile([128, SC], F32, tag="pu", name="pbc")
                    nc.tensor.matmul(st["pbc"][:], lhsT=inv2[64:128, :],
                                     rhs=st["odA"][64:128, :],
                                     start=True, stop=False)
                    nc.tensor.matmul(st["pbc"][:], lhsT=inv2[0:64, :],
                                     rhs=st["odB"][0:64, :],
                                     start=False, stop=True)
                    st["invd"] = work.tile([128, QC], F32, tag="invd", name="invd")
                    nc.vector.reciprocal_approx_fast(st["invd"][:], st["pbc"][:])
                elif stage == 2:
                    st["ot"] = work.tile([128, QC], F16, tag="ot", name="ot")
                    nc.vector.tensor_mul(out=st["ot"][0:64, :],
                                         in0=st["odA"][0:64, :].bitcast(F32),
                                         in1=st["invd"][0:64, :])
                    nc.vector.tensor_mul(out=st["ot"][64:128, :],
                                         in0=st["odB"][64:128, :].bitcast(F32),
                                         in1=st["invd"][64:128, :])
                else:
                    # stages 3..6: output projection, one 128-row chunk each.
                    # On the final (unpipelined) tail, use the then-free
                    # scores PSUM banks so matmuls never wait on the drains,
                    # and split the drains with the then-idle ACT engine.
                    j = stage - 3
                    tail = st.get("act_copies", False)
                    osb = osbp.tile([128, D], F16, tag="osb", name="osb")
                    if tail:
                        # final unpipelined tail: use the then-free scores
                        # PSUM banks so matmuls never wait on the drains
                        po = ps_sc.tile([128, 2 * SC], F32, tag="pss", name="po2")
                        for e in range(D // SC):
                            nc.tensor.matmul(
                                po[:, e * SC:(e + 1) * SC],
                                lhsT=st["ot"][:, j * 128:(j + 1) * 128],
                                rhs=woutT[:, e * SC:(e + 1) * SC],
                                start=True, stop=True)
                        nc.scalar.copy(osb[:, 0:SC], po[:, 0:SC])
                        nc.vector.tensor_copy(osb[:, SC:2 * SC], po[:, SC:2 * SC])
                    else:
                        for e in range(D // SC):
                            po = ps_util.tile([128, SC], F32, tag="pu", name="po")
                            nc.tensor.matmul(
                                po[:], lhsT=st["ot"][:, j * 128:(j + 1) * 128],
                                rhs=woutT[:, e * SC:(e + 1) * SC],
                                start=True, stop=True)
                            # last chunk's drains go to ACT (idle near the
                            # iteration boundary) so DVE is clear for the
                            # next iteration's accumulator drains
                            if stage == 6:
                                nc.scalar.copy(osb[:, e * SC:(e + 1) * SC],
                                               po[:])
                            else:
                                nc.vector.tensor_copy(osb[:, e * SC:(e + 1) * SC],
                                                      po[:])
                    row = st["q0"] + j * 128
                    nc.sync.dma_start(out_d[row:row + 128, :], osb[:])

            # finish stages of the previous iteration land at these k slots
            # (drains run at the loop top); out-proj stages 3-6 sit so the
            # DVE finishes their PSUM drains well before the next
            # iteration's accumulator drains need the engine.
            FIN_AT = {3: 1, 5: 2, 7: 3, 9: 4, 11: 5, 13: 6}

            attn_ctx = {}

            def emit_attn_range(b, q, prev, klo, khi, filler=None):
                """Chunks [klo, khi) of one (batch, q-chunk) iteration.
                attnV lags scores by 2 chunks so its exp-wait is always
                pre-cleared and LDWEIGHTS prefetch is never blocked."""
                filler = filler or {}
                q0 = b * S + q * QC
                if klo == 0:
                    attn_ctx["psA"] = ps_acc.tile([128, SC], F32, tag="psA",
                                                  name="psA")
                    attn_ctx["psB"] = ps_acc.tile([128, SC], F32, tag="psB",
                                                  name="psB")
                    attn_ctx["pend"] = []
                    emit_finish_stage(prev, 0)      # drains first on DVE
                psA, psB, pend = (attn_ctx["psA"], attn_ctx["psB"],
                                  attn_ctx["pend"])

                def emit_attnv(k, eb):
                    nc.tensor.matmul(psA[:], lhsT=vaug[b][0][:, k, :],
                                     rhs=eb[:, 0:QC],
                                     start=(k == 0), stop=(k == NKC - 1))
                    nc.tensor.matmul(psB[:], lhsT=vaug[b][1][:, k, :],
                                     rhs=eb[:, QC:2 * QC],
                                     start=(k == 0), stop=(k == NKC - 1))

                for k in range(klo, khi):
                    kcol = b * S + k * 128
                    pss = ps_sc.tile([128, 2 * QC], F32, tag="pss", name="pss")
                    # scores^T, two heads packed in PE row groups (the pair
                    # executes concurrently on disjoint row halves)
                    nc.tensor.matmul(
                        pss[:, 0:QC], lhsT=KT[0:64, kcol:kcol + 128],
                        rhs=QT[0:64, q0:q0 + QC], start=True, stop=True)
                    nc.tensor.matmul(
                        pss[:, QC:2 * QC], lhsT=KT[64:128, kcol:kcol + 128],
                        rhs=QT[64:128, q0:q0 + QC], start=True, stop=True)
                    # pop attnV in pairs on odd chunks: halves the number of
                    # row-group->full-width PE transitions (each pays a
                    # ~100ns array-drain wait on the first full-width MM).
                    # Finish-stage matmuls go after the pops for the same
                    # reason.
                    if k % 2 == 1:
                        while len(pend) > 2:
                            emit_attnv(*pend.pop(0))
                    if k in FIN_AT:
                        emit_finish_stage(prev, FIN_AT[k])
                    eb = epool.tile([128, 2 * QC], BF16, tag="eb", name="eb")
                    nc.scalar.activation(eb[:], pss[:], AF.Exp, scale=float(SCALE))
                    pend.append((k, eb))
                    for fn in filler.get(k, ()):
                        fn()
                if khi == NKC:
                    while pend:
                        emit_attnv(*pend.pop(0))
                    return {"q0": q0, "psA": psA, "psB": psB}
                return prev

            def K_(s):
                return lambda: emit_qkv_part(s, 1)

            def V_(s):
                return lambda: emit_qkv_part(s, 2)

            def Q_(s):
                return lambda: emit_qkv_part(s, 0)

            def T_(s):
                return lambda: emit_vtrans(s)

            # ---- emission order.  Only K/V/vaug of b0's first s-chunk (plus
            # Q of s0) are needed to start attention; everything else spreads
            # through the k-loops as PE filler, each block emitted before its
            # first consumer. ----
            for fn in (K_(0), Q_(0), V_(0), T_(0)):
                fn()
            prev = None
            prev = emit_attn_range(0, 0, prev, 0, 8, {
                1: (K_(1), V_(1)), 3: (T_(1),), 5: (K_(2),),
                7: (V_(2),)})
            prev = emit_attn_range(0, 0, prev, 8, 16, {
                9: (T_(2), K_(3)), 11: (V_(3),), 13: (T_(3),),
                15: (Q_(1),)})
            prev = emit_attn_range(0, 1, prev, 0, 16, {
                3: (K_(4),), 7: (V_(4),), 11: (T_(4),), 13: (Q_(2),)})
            prev = emit_attn_range(0, 2, prev, 0, 16, {
                3: (K_(5),), 7: (V_(5),), 5: (K_(6),), 11: (T_(5),),
                13: (Q_(3),)})
            prev = emit_attn_range(0, 3, prev, 0, 16, {
                3: (V_(6),), 5: (K_(7),), 7: (V_(7),), 9: (T_(6),),
                11: (T_(7),), 13: (Q_(4),)})
            prev = emit_attn_range(1, 0, prev, 0, 16, {13: (Q_(5),)})
            prev = emit_attn_range(1, 1, prev, 0, 16, {13: (Q_(6),)})
            prev = emit_attn_range(1, 2, prev, 0, 16, {13: (Q_(7),)})
            prev = emit_attn_range(1, 3, prev, 0, 16)
            prev["act_copies"] = True
            for stage in range(7):
                emit_finish_stage(prev, stage)

    nc.compile()
    return nc


def _get_nc():
    if "nc" not in _cache:
        _cache["nc"] = _build()
    return _cache["nc"]


def _prep_inputs(x, w_qkv, w_out):
    import ml_dtypes
    bf16 = ml_dtypes.bfloat16
    x = np.asarray(x, dtype=np.float32)
    w_qkv = np.asarray(w_qkv, dtype=np.float32)
    w_out = np.asarray(w_out, dtype=np.float32)
    # xT[pi, s, po, c] = x^T[po*128+pi, s*512+c]: every device tile DMA is
    # 128 contiguous per-partition descriptors
    xT = x.reshape(BS, D).T.astype(bf16)
    xT = np.ascontiguousarray(
        xT.reshape(DC, 128, BS // SC, SC).transpose(1, 2, 0, 3))
    in_maps = []
    for c in range(N_CORES):
        # reference splits qkv as (v, q, k): v rows [0,D), q [D,2D), k [2D,3D)
        wq = w_qkv[D + 128 * c: D + 128 * (c + 1)]
        wk = w_qkv[2 * D + 128 * c: 2 * D + 128 * (c + 1)]
        wv = w_qkv[128 * c: 128 * (c + 1)]
        wqkvT = np.concatenate([wq, wk, wv], axis=0).T.astype(bf16)  # [D, 384]
        wqkvT = np.ascontiguousarray(
            wqkvT.reshape(DC, 128, 384).transpose(1, 0, 2))
        woutT = np.ascontiguousarray(
            w_out[:, 128 * c:128 * (c + 1)].T.astype(np.float16))
        in_maps.append({"xT": xT, "wqkvT": wqkvT, "woutT": woutT})
    return in_maps


def kernel(x, w_qkv, w_out, b_out):
    from concourse.bass_utils import run_bass_kernel_spmd

    nc = _get_nc()
    in_maps = _prep_inputs(x, w_qkv, w_out)
    b_out = np.asarray(b_out, dtype=np.float32)
    res = run_bass_kernel_spmd(nc, in_maps, core_ids=list(range(N_CORES)))
    acc = np.zeros((BS, D), np.float32)
    for c in range(N_CORES):
        acc += res.results[c]["out"].astype(np.float32)
    acc = acc + b_out[None, :]
    return acc.reshape(B, S, D)
